# revision 6
# baseline (speedup 1.0000x reference)
"""Trainium2 Bass kernel for NeuralSumProductModel (LDPC sum-product decoder).

Contract: kernel(**inputs) takes FULL inputs (llr [512,8192] f32,
var_index [24576] i32, chk_index [24576] i32) and returns the FULL
output [5, 512, 8192] f32, matching reference.reference().

Design (per NeuronCore, batch sharded 512 -> 8 x 64):
  - partitions = (edge-half h, batch b): p = h*64 + b
  - edges in check-major order (sorted by check, 6 per check); half h owns
    checks [h*2048,(h+1)*2048) = edge cols [h*12288,(h+1)*12288)
  - one big SBUF gather TABLE [128, 45056] f32:
      [locA 0:12288 | foreign 12288:24576 | locB 24576:36864 | x 36864:45056]
    ext state ping-pongs between locA/locB by iteration parity so gathers of
    the old state never conflict with writes of the new state. 'foreign' is
    cross-filled by DMA from the partner partition half each iteration.
  - var-side ops are GPSIMD ap_gather's: msg_e = xs_e + ext[sib1] + ext[sib2]
    (siblings share e's variable), out_v = x_v + sum of ext at v's 3 edges.
  - check-side ops are strided free-axis DVE/ACT ops over groups of 6.
  - magnitude via phi involution: ext_mag = -ln(max(tanh(-d/2), TCLIP)),
    exactly 2*atanh(min(e^d, 1-1e-7)); sign via float sign-product tree.
"""

import os
import sys

import numpy as np

for _p in ("/opt/trn_rl_repo", "/root/.axon_site/_ro/trn_rl_repo"):
    if os.path.isdir(_p) and _p not in sys.path:
        sys.path.insert(0, _p)

N_VAR, N_CHK, DV, DC = 8192, 4096, 3, 6
E = N_VAR * DV  # 24576
BATCH, N_ITER, N_CORES = 512, 5, 8
BC = BATCH // N_CORES           # 64 batch rows per core
HE = E // 2                     # 12288 edge cols per half
HC = N_CHK // 2                 # 2048 checks per half
HV = N_VAR // 2                 # 4096 vars per half
N_ECH = 16                      # check chunks per iteration
ECH = HE // N_ECH               # 768 edge cols per chunk
CCH = ECH // DC                 # 128 checks per chunk
N_VCH = 16                      # var chunks
VCH = HV // N_VCH               # 256 vars per chunk
T_LOCA, T_FOR, T_LOCB, T_X = 0, HE, 2 * HE, 3 * HE
T_COLS = 3 * HE + N_VAR         # 45056
WIN = 2 * HE                    # 24576-col sib gather window

EPS = 1e-12
_C = np.float32(1.0) - np.float32(1e-7)
TCLIP = float(np.float32((np.float32(1.0) - _C) / (np.float32(1.0) + _C)))

_CACHE = {}
_LAST_RESULTS = None


def _wrap(stream):
    """Pack an unwrapped per-core index stream [8, n] -> wrapped [128, n//16].

    ap_gather unwraps core k's indices as unwrapped[s*16+p] = tile[16k+p, s].
    """
    st = np.asarray(stream, np.int16)
    ncore, n = st.shape
    assert n % 16 == 0
    out = np.zeros((16 * ncore, n // 16), np.int16)
    for k in range(ncore):
        out[16 * k:16 * (k + 1), :] = st[k].reshape(n // 16, 16).T
    return out


def _build_indices(vi, ci):
    """Host-side graph preprocessing. Returns dict of wrapped index planes."""
    order = np.argsort(ci, kind="stable")          # check-major edge list
    cm_var = vi[order].astype(np.int64)            # var of each cm edge
    pos_of_edge = np.empty(E, np.int64)
    pos_of_edge[order] = np.arange(E)
    edges_of_var = np.argsort(vi, kind="stable").reshape(N_VAR, DV)
    pos_var = pos_of_edge[edges_of_var]            # [N_VAR, 3] cm positions

    half_of_pos = pos_var // HE                    # [N_VAR, 3]

    def rel(p, H, parity):
        # relative coord of global cm position p within the sib window of
        # `parity` (0 = A window [0:24576), 1 = B window [12288:36864)),
        # as seen from a partition in half H.
        same = (p // HE) == H
        if parity == 0:
            return (p % HE) + HE * (~same)
        return (p % HE) + HE * same

    planes = {}
    # sibling + x index streams, per half
    for parity in (0, 1):
        s1 = np.zeros((2, HE), np.int64)
        s2 = np.zeros((2, HE), np.int64)
        for H in (0, 1):
            jj = np.arange(H * HE, (H + 1) * HE)
            v = cm_var[jj]                          # [HE]
            pv = pos_var[v]                         # [HE, 3]
            # sibling positions: the 2 of pv != jj, kept in slot order
            mask = pv != jj[:, None]
            sib = pv[mask].reshape(HE, 2)
            s1[H] = rel(sib[:, 0], H, parity)
            s2[H] = rel(sib[:, 1], H, parity)
        planes[f"s1{'ab'[parity]}"] = _wrap(
            np.concatenate([np.repeat(s1[0][None], 4, 0),
                            np.repeat(s1[1][None], 4, 0)]))
        planes[f"s2{'ab'[parity]}"] = _wrap(
            np.concatenate([np.repeat(s2[0][None], 4, 0),
                            np.repeat(s2[1][None], 4, 0)]))
    xi = np.zeros((2, HE), np.int64)
    for H in (0, 1):
        xi[H] = cm_var[np.arange(H * HE, (H + 1) * HE)]
    planes["xi"] = _wrap(np.concatenate([np.repeat(xi[0][None], 4, 0),
                                         np.repeat(xi[1][None], 4, 0)]))

    # out gathers: var v (local to half H) -> its 3 edge positions
    for parity in (0, 1):
        for s in range(DV):
            vg = np.zeros((2, HV), np.int64)
            for H in (0, 1):
                vids = np.arange(H * HV, (H + 1) * HV)
                vg[H] = rel(pos_var[vids, s], H, parity)
            planes[f"vg{s}{'ab'[parity]}"] = _wrap(
                np.concatenate([np.repeat(vg[0][None], 4, 0),
                                np.repeat(vg[1][None], 4, 0)]))
    return planes


def _build_bass():
    import concourse.bass as bass
    import concourse.tile as tile
    from concourse import bacc, mybir
    from contextlib import ExitStack

    dt = mybir.dt
    F32, I16 = dt.float32, dt.int16
    ALU = mybir.AluOpType
    ACT = mybir.ActivationFunctionType
    AX = mybir.AxisListType

    nc = bacc.Bacc("TRN2", target_bir_lowering=False, debug=False)

    llr_d = nc.dram_tensor("llr", [BC, N_VAR], F32, kind="ExternalInput").ap()
    idx_d = {}
    for nm in ("s1a", "s1b", "s2a", "s2b", "xi"):
        idx_d[nm] = nc.dram_tensor(nm, [128, HE // 16], I16,
                                   kind="ExternalInput").ap()
    for parity in (0, 1):
        for s in range(DV):
            nm = f"vg{s}{'ab'[parity]}"
            idx_d[nm] = nc.dram_tensor(nm, [128, HV // 16], I16,
                                       kind="ExternalInput").ap()
    out_d = nc.dram_tensor("out", [N_ITER, BC, N_VAR], F32,
                           kind="ExternalOutput").ap()

    with tile.TileContext(nc) as tc, ExitStack() as ctx:
        big = ctx.enter_context(tc.tile_pool(name="big", bufs=1))
        wp = ctx.enter_context(tc.tile_pool(name="wp", bufs=1))
        pp = ctx.enter_context(tc.tile_pool(name="pp", bufs=1, space="PSUM"))

        table = big.tile([128, T_COLS], F32, tag="table")
        # persistent smalls: csum|cp1|cp|p3|eps packed in one 4KB tile
        sm = big.tile([128, 772], F32, tag="smalls")
        sm_csum = sm[:, 0:CCH]
        sm_cp1 = sm[:, CCH:2 * CCH]
        sm_cp = sm[:, 2 * CCH:3 * CCH]
        sm_p3 = sm[:, 3 * CCH:6 * CCH]
        sm_eps = sm[:, 768:769]
        nc.vector.memset(sm_eps, EPS)

        # load x region (duplicated across halves)
        nc.sync.dma_start(table[0:64, T_X:T_X + N_VAR], llr_d[:, :])
        nc.sync.dma_start(table[64:128, T_X:T_X + N_VAR], llr_d[:, :])

        xwin = table[:, T_X:T_X + N_VAR]
        IC = ECH // 16          # wrapped idx cols per check chunk (48)
        IVC = VCH // 16         # wrapped idx cols per var chunk (16)

        for it in range(N_ITER):
            side = it % 2
            wr = T_LOCA if side == 0 else T_LOCB
            wloc = table[:, wr:wr + HE]

            # per-iteration index tile: s1|s2|xi|vg0|vg1|vg2
            ixt = wp.tile([128, 3072], I16, tag="idx")
            pab = "ab"[(it - 1) % 2]
            cab = "ab"[side]
            if it > 0:
                nc.sync.dma_start(ixt[:, 0:768], idx_d[f"s1{pab}"][:])
                nc.sync.dma_start(ixt[:, 768:1536], idx_d[f"s2{pab}"][:])
            nc.sync.dma_start(ixt[:, 1536:2304], idx_d["xi"][:])
            for s in range(DV):
                nc.sync.dma_start(ixt[:, 2304 + 256 * s:2304 + 256 * (s + 1)],
                                  idx_d[f"vg{s}{cab}"][:])
            ix_s1 = ixt[:, 0:768]
            ix_s2 = ixt[:, 768:1536]
            ix_xi = ixt[:, 1536:2304]

            if it > 0:
                pwin_off = T_LOCA if (it - 1) % 2 == 0 else T_FOR
                pwin = table[:, pwin_off:pwin_off + WIN]

            for c in range(N_ECH):
                cl = slice(c * ECH, (c + 1) * ECH)
                ic = slice(c * IC, (c + 1) * IC)
                G = wp.tile([128, 3 * ECH], F32, tag="G")
                g1, g2, g3 = G[:, 0:ECH], G[:, ECH:2 * ECH], G[:, 2 * ECH:3 * ECH]
                nc.gpsimd.ap_gather(g3, xwin, ix_xi[:, ic],
                                    channels=128, num_elems=N_VAR, d=1,
                                    num_idxs=ECH)
                if it == 0:
                    msg_ap = g3
                else:
                    nc.gpsimd.ap_gather(g1, pwin, ix_s1[:, ic],
                                        channels=128, num_elems=WIN, d=1,
                                        num_idxs=ECH)
                    nc.gpsimd.ap_gather(g2, pwin, ix_s2[:, ic],
                                        channels=128, num_elems=WIN, d=1,
                                        num_idxs=ECH)
                    pm = pp.tile([128, ECH], F32, tag="P1")
                    nc.vector.tensor_tensor(pm[:], g1, g2, op=ALU.add)
                    msg = pp.tile([128, ECH], F32, tag="P2")
                    nc.vector.tensor_tensor(msg[:], pm[:], g3, op=ALU.add)
                    msg_ap = msg[:]

                tt = pp.tile([128, ECH], F32, tag="P1")
                nc.scalar.activation(tt[:], msg_ap, ACT.Tanh, scale=0.5)
                sg = wp.tile([128, ECH], F32, tag="sgx")
                nc.scalar.activation(sg[:], tt[:], ACT.Sign)
                ab = pp.tile([128, ECH], F32, tag="P2")
                nc.scalar.activation(ab[:], tt[:], ACT.Abs)
                la = pp.tile([128, ECH], F32, tag="P3")
                nc.scalar.activation(la[:], ab[:], ACT.Ln, bias=sm_eps)

                la6 = la[:].rearrange("p (n k) -> p n k", k=DC)
                nc.vector.tensor_reduce(sm_csum, la6, axis=AX.X, op=ALU.add)

                sg6 = sg[:].rearrange("p (n k) -> p n k", k=DC)
                p3v = sm_p3.rearrange("p (n k) -> p n k", k=3)
                nc.vector.tensor_tensor(p3v, sg6[:, :, 0:3], sg6[:, :, 3:6],
                                        op=ALU.mult)
                nc.vector.tensor_tensor(sm_cp1, p3v[:, :, 0], p3v[:, :, 1],
                                        op=ALU.mult)
                nc.vector.tensor_tensor(sm_cp, sm_cp1, p3v[:, :, 2],
                                        op=ALU.mult)

                dd = pp.tile([128, ECH], F32, tag="P4")
                dd6 = dd[:].rearrange("p (n k) -> p n k", k=DC)
                csb = sm_csum.unsqueeze(2).broadcast_to([128, CCH, DC])
                nc.vector.tensor_tensor(dd6, csb, la6, op=ALU.subtract)

                t2 = pp.tile([128, ECH], F32, tag="P1")
                nc.scalar.activation(t2[:], dd[:], ACT.Tanh, scale=-0.5)
                t2c = pp.tile([128, ECH], F32, tag="P2")
                nc.vector.tensor_scalar_max(t2c[:], t2[:], TCLIP)

                se = pp.tile([128, ECH], F32, tag="P4")
                se6 = se[:].rearrange("p (n k) -> p n k", k=DC)
                cpb = sm_cp.unsqueeze(2).broadcast_to([128, CCH, DC])
                nc.vector.tensor_tensor(se6, sg6, cpb, op=ALU.mult)

                a5 = wp.tile([128, ECH], F32, tag="sgx")
                nc.scalar.activation(a5[:], t2c[:], ACT.Ln)

                # ext = (-a5) * se  ->  local write side
                nc.vector.scalar_tensor_tensor(
                    wloc[:, cl], a5[:], -1.0, se[:],
                    op0=ALU.mult, op1=ALU.mult)

            # cross-fill to the partner half's foreign region. Emitted after
            # ALL of this iteration's sibling gathers so they still read the
            # previous state's foreign values (the region is single-buffered).
            for c in range(N_ECH):
                cl = slice(c * ECH, (c + 1) * ECH)
                nc.sync.dma_start(
                    table[0:64, T_FOR:T_FOR + HE][:, cl],
                    wloc[64:128, cl])
                nc.sync.dma_start(
                    table[64:128, T_FOR:T_FOR + HE][:, cl],
                    wloc[0:64, cl])

            # out phase: reads current state window
            cwin_off = T_LOCA if side == 0 else T_FOR
            cwin = table[:, cwin_off:cwin_off + WIN]
            for vc in range(N_VCH):
                iv = slice(vc * IVC, (vc + 1) * IVC)
                geo = wp.tile([128, 3 * VCH], F32, tag="G")
                ge = [geo[:, s * VCH:(s + 1) * VCH] for s in range(DV)]
                for s in range(DV):
                    nc.gpsimd.ap_gather(
                        ge[s], cwin, ixt[:, 2304 + 256 * s:2304 + 256 * (s + 1)][:, iv],
                        channels=128, num_elems=WIN, d=1, num_idxs=VCH)
                vso = wp.tile([128, 2 * VCH], F32, tag="sgx")
                vs, ov = vso[:, 0:VCH], vso[:, VCH:2 * VCH]
                nc.vector.tensor_tensor(vs, ge[0], ge[1], op=ALU.add)
                nc.vector.tensor_tensor(ov, vs, ge[2], op=ALU.add)
                vl = slice(vc * VCH, (vc + 1) * VCH)
                nc.vector.tensor_tensor(
                    vs[0:64], ov[0:64],
                    table[0:64, T_X:T_X + HV][:, vl], op=ALU.add)
                nc.vector.tensor_tensor(
                    vs[64:128], ov[64:128],
                    table[64:128, T_X + HV:T_X + N_VAR][:, vl], op=ALU.add)
                nc.sync.dma_start(out_d[it, :, vc * VCH:(vc + 1) * VCH],
                                  vs[0:64])
                nc.sync.dma_start(
                    out_d[it, :, HV + vc * VCH:HV + (vc + 1) * VCH],
                    vs[64:128])

    nc.compile()
    return nc


def _numpy_fallback(llr, vi, ci):
    x = llr.T.astype(np.float32)
    scattered = x[vi]
    ext = np.zeros_like(scattered)
    outs = []
    for _ in range(N_ITER):
        vsum = np.zeros((N_VAR, x.shape[1]), np.float32)
        np.add.at(vsum, vi, ext)
        msg = (vsum[vi] - ext) + scattered
        t = np.tanh(msg * 0.5)
        la = np.log(np.abs(t) + EPS)
        sg = np.sign(t)
        cs = np.zeros((N_CHK, x.shape[1]), np.float32)
        np.add.at(cs, ci, la)
        cpr = np.ones((N_CHK, x.shape[1]), np.float32)
        np.multiply.at(cpr, ci, sg)
        loo = np.exp(cs[ci] - la) * (cpr[ci] * sg)
        loo = np.clip(loo, -float(_C), float(_C))
        ext = 2.0 * np.arctanh(loo)
        vs2 = np.zeros((N_VAR, x.shape[1]), np.float32)
        np.add.at(vs2, vi, ext)
        outs.append((vs2 + x).T)
    return np.stack(outs)


def kernel(llr, var_index, chk_index):
    llr = np.asarray(llr, np.float32)
    vi = np.asarray(var_index, np.int64).ravel()
    ci = np.asarray(chk_index, np.int64).ravel()
    assert llr.shape == (BATCH, N_VAR) and vi.shape == (E,) and ci.shape == (E,)

    regular = (np.array_equal(np.bincount(vi, minlength=N_VAR),
                              np.full(N_VAR, DV))
               and np.array_equal(np.bincount(ci, minlength=N_CHK),
                                  np.full(N_CHK, DC)))
    if not regular:
        return _numpy_fallback(llr, vi, ci).astype(np.float32)

    key = ("k", hash(vi.tobytes()), hash(ci.tobytes()))
    if key not in _CACHE:
        planes = _build_indices(vi, ci)
        nc = _build_bass()
        _CACHE[key] = (nc, planes)
    nc, planes = _CACHE[key]

    from concourse.bass_utils import run_bass_kernel_spmd
    in_maps = []
    for c in range(N_CORES):
        m = {nm: np.ascontiguousarray(v) for nm, v in planes.items()}
        m["llr"] = np.ascontiguousarray(llr[c * BC:(c + 1) * BC, :])
        in_maps.append(m)
    trace = os.environ.get("BASS_KERNEL_TRACE", "0") == "1"
    res = run_bass_kernel_spmd(nc, in_maps, list(range(N_CORES)), trace=trace)
    global _LAST_RESULTS
    _LAST_RESULTS = res
    out = np.concatenate([res.results[c]["out"] for c in range(N_CORES)],
                         axis=1)
    return np.ascontiguousarray(out, dtype=np.float32)


if __name__ == "__main__":
    sys.path.insert(0, os.path.dirname(os.path.abspath(__file__)))
    import reference
    inputs = {k: np.asarray(v) for k, v in reference.setup_inputs().items()}
    exp = np.asarray(reference.reference(**inputs))
    got = kernel(**inputs)
    err = np.max(np.abs(got - exp)) / (np.max(np.abs(exp)) + 1e-30)
    print("Relative error:", err)
